# revision 5
# baseline (speedup 1.0000x reference)
"""DenseAttention (causal quadratic variant, no softmax) — TRN2 Bass kernel.

Problem: out[b] = (tril(Q @ K^T) @ V) per head, where
  Q = X @ Wq (split into 16 heads of 64), K = V = X head slices.
Shapes: X [2, 2048, 1024] fp32, Wq [1024, 1024] fp32 -> out [2, 2048, 1024] fp32.

Sharding (8 cores): core c -> batch b = c//4, head group g = c%4 (4 heads,
output columns [256g, 256g+256)).  The queries projection is column-sharded
by head group; no cross-device communication.

Algorithm per core (linear-attention prefix-sum form, per head h):
  attn_I = Q_I @ S_{<I} + tril(Q_I @ K_I^T) @ V_I      (blocks I of 256 rows)
  S_I = S_{<I} + sum over 128-blocks j in I of K_j^T @ V_j   ([64,64] state)
This reduces the strictly-causal off-diagonal work from O(N^2 hd) to O(N hd^2).
Everything is computed transposed (attnT [hd, N]) so both matmul stages feed
the tensor engine without any on-device transposes; the host un-transposes.

All matmuls run in bf16 with fp32 PSUM accumulation (validated ~2.8e-3 rel
error vs the fp32 reference in numpy emulation).
"""

import numpy as np
import ml_dtypes

import concourse.bacc as bacc
import concourse.mybir as mybir
import concourse.tile as tile
from concourse import bass_utils
from concourse.bass import ds, ts

B, N, D = 2, 2048, 1024
H, HD = 16, 64
NCORES = 8
P = 128           # partition dim
NQ = 256          # q-block (outer) size
T = N // NQ       # 8 outer blocks
KB = N // P       # 16 k-blocks
CW = 256          # per-core output column width (4 heads)

DT = mybir.dt.bfloat16
NPDT = ml_dtypes.bfloat16
F32 = mybir.dt.float32


def _emit(nc, tc, xt_d, wq_d, xv_d, mk_d, out_d):
    with (
        tc.tile_pool(name="const", bufs=1) as cpool,
        tc.tile_pool(name="work", bufs=4) as wpool,
        tc.tile_pool(name="psq", bufs=1, space="PSUM") as psq,
        tc.tile_pool(name="pss", bufs=1, space="PSUM") as pss,
        tc.tile_pool(name="psst", bufs=2, space="PSUM") as psst,
        tc.tile_pool(name="psat", bufs=2, space="PSUM") as psat,
    ):
        # ---------------- input DMAs (xv/wq first so S-phase + Q-proj can start early)
        mk_sb = cpool.tile([P, 2 * NQ], DT, name="mk_sb", tag="mk_sb")
        nc.sync.dma_start(out=mk_sb, in_=mk_d)

        xv_sb = []
        for j in range(KB):
            tl = cpool.tile([P, CW], DT, name=f"xv{j}", tag=f"xv{j}")
            nc.sync.dma_start(out=tl, in_=xv_d[ts(j, P), :])
            xv_sb.append(tl)

        wq_sb = []
        for k in range(8):
            tl = cpool.tile([P, CW], DT, name=f"wq{k}", tag=f"wq{k}")
            nc.sync.dma_start(out=tl, in_=wq_d[ts(k, P), :])
            wq_sb.append(tl)

        xt_sb = []
        for k in range(8):
            tl = cpool.tile([P, N], DT, name=f"xt{k}", tag=f"xt{k}")
            nc.sync.dma_start(out=tl, in_=xt_d[ts(k, P), :])
            xt_sb.append(tl)

        # ---------------- S phase: running prefix sums S_t = sum_{j<=2t+1} K_j^T V_j
        # Head pair p: even head at psum partitions 0-63, odd head at 64-127.
        # Snapshots after each outer block t (only t=0..6 are consumed, by t+1).
        ssb = [[None] * (T - 1) for _ in range(2)]
        for p in range(2):
            sps = pss.tile([P, HD], F32, name=f"sps{p}", tag=f"sps{p}")
            for j in range(KB):
                for e in range(2):
                    v = xv_sb[j][:, ds(P * p + HD * e, HD)]
                    nc.tensor.matmul(
                        sps[ds(HD * e, HD), :], v, v,
                        start=(j == 0), stop=(j == KB - 1),
                        tile_position=(0, HD * e),
                        # sim's coarse group check mishandles base_partition
                        # 64 slices; per-partition has_written semantics are
                        # correct (validated in CoreSim)
                        skip_group_check=True,
                    )
                if j % 2 == 1 and j < KB - 1:
                    t_idx = j // 2
                    snap = cpool.tile(
                        [P, HD], DT, name=f"ssb{p}_{t_idx}", tag=f"ssb{p}_{t_idx}"
                    )
                    nc.vector.tensor_copy(snap, sps)
                    ssb[p][t_idx] = snap

        # ---------------- Q projection: QT = Wq^T @ XT, [256, 2048] as 2 x [128, 2048]
        # k (contraction over D) outer so compute chases the xt DMA stream.
        qt_sb = [
            cpool.tile([P, N], DT, name=f"qt{m}", tag=f"qt{m}") for m in range(2)
        ]
        for m in range(2):
            for half in range(2):
                qp = [
                    psq.tile([P, 512], F32, name=f"qp{m}_{half}_{i}", tag=f"qp{i}")
                    for i in range(2)
                ]
                for k in range(8):
                    for i in range(2):
                        nchunk = 2 * half + i
                        nc.tensor.matmul(
                            qp[i],
                            wq_sb[k][:, ds(P * m, P)],
                            xt_sb[k][:, ds(512 * nchunk, 512)],
                            start=(k == 0), stop=(k == 7),
                        )
                for i in range(2):
                    nchunk = 2 * half + i
                    nc.scalar.copy(qt_sb[m][:, ds(512 * nchunk, 512)], qp[i])

        # ---------------- attention: per head pair p, per outer q-block t
        for p in range(2):
            for t in range(T):
                at = psat.tile([P, NQ], F32, name=f"at{p}_{t}", tag="at")

                # diagonal-block scores ST_j = K_j @ Q_t^T  [128 krows, 256 qcols]
                st_list = []
                for o in range(2):
                    j = 2 * t + o
                    for e in range(2):
                        stp = psst.tile(
                            [P, NQ], F32, name=f"stp{p}_{t}_{o}_{e}", tag="stp"
                        )
                        kt = xt_sb[p][ds(HD * e, HD), ds(P * j, P)]
                        qv = qt_sb[p][ds(HD * e, HD), ds(NQ * t, NQ)]
                        nc.tensor.matmul(stp, kt, qv, start=True, stop=True)
                        stsb = wpool.tile(
                            [P, NQ], DT, name=f"st{p}_{t}_{o}_{e}", tag="st"
                        )
                        # causal mask fused into the PSUM->SBUF copy
                        nc.vector.tensor_mul(stsb, stp, mk_sb[:, ds(NQ * o, NQ)])
                        st_list.append((o, e, stsb))

                # global term: attnT_t += S_{<t}^T @ Q_t^T (S symmetric; use as-is)
                for e in range(2):
                    if t > 0:
                        nc.tensor.matmul(
                            at[ds(HD * e, HD), :],
                            ssb[p][t - 1][ds(HD * e, HD), :],
                            qt_sb[p][ds(HD * e, HD), ds(NQ * t, NQ)],
                            start=True, stop=False,
                            tile_position=(HD * e, HD * e),
                            skip_group_check=True,
                        )

                # diagonal term: attnT_t += V_j^T @ ST_j
                for o, e, stsb in st_list:
                    j = 2 * t + o
                    nc.tensor.matmul(
                        at[ds(HD * e, HD), :],
                        xv_sb[j][:, ds(P * p + HD * e, HD)],
                        stsb,
                        start=(t == 0 and o == 0), stop=(o == 1),
                        tile_position=(0, HD * e),
                        skip_group_check=True,
                    )

                ot = wpool.tile([P, NQ], F32, name=f"ot{p}_{t}", tag="ot")
                nc.scalar.copy(ot, at)
                nc.sync.dma_start(
                    out=out_d[ds(P * p, P), ds(NQ * t, NQ)], in_=ot
                )


def build_nc(loop_n=1):
    nc = bacc.Bacc("TRN2", target_bir_lowering=False, debug=False)
    xt_d = nc.dram_tensor("xt", [D, N], DT, kind="ExternalInput").ap()
    wq_d = nc.dram_tensor("wq", [D, CW], DT, kind="ExternalInput").ap()
    xv_d = nc.dram_tensor("xv", [N, CW], DT, kind="ExternalInput").ap()
    mk_d = nc.dram_tensor("mk", [P, 2 * NQ], DT, kind="ExternalInput").ap()
    out_d = nc.dram_tensor("outT", [CW, N], F32, kind="ExternalOutput").ap()
    with tile.TileContext(nc) as tc:
        if loop_n > 1:
            # timing-only build: repeat the whole kernel on-device so the
            # per-iteration time can be separated from host/RPC overhead
            with tc.For_i(0, loop_n, 1):
                _emit(nc, tc, xt_d, wq_d, xv_d, mk_d, out_d)
        else:
            _emit(nc, tc, xt_d, wq_d, xv_d, mk_d, out_d)
    nc.compile()
    return nc


_CACHE = {}


def get_nc():
    if "nc" not in _CACHE:
        _CACHE["nc"] = build_nc()
    return _CACHE["nc"]


def make_in_maps(hidden_states, queries_weight):
    X = np.asarray(hidden_states, dtype=np.float32)
    W = np.asarray(queries_weight, dtype=np.float32)
    r = np.arange(P)[:, None]
    c = np.arange(NQ)[None, :]
    m0 = (c >= r).astype(np.float32)
    m1 = (c >= r + P).astype(np.float32)
    mk = np.concatenate([m0, m1], axis=1).astype(NPDT)
    in_maps = []
    for core in range(NCORES):
        b, g = divmod(core, 4)
        cols = slice(CW * g, CW * g + CW)
        # Permute the contraction rows so every core sees its own heads'
        # K^T rows at xt rows [0, 256) (keeps the program core-agnostic).
        perm = np.r_[
            np.arange(CW * g, CW * g + CW),
            np.arange(0, CW * g),
            np.arange(CW * g + CW, D),
        ]
        in_maps.append({
            "xt": np.ascontiguousarray(X[b].T[perm]).astype(NPDT),
            "wq": np.ascontiguousarray(W[perm][:, cols]).astype(NPDT),
            "xv": np.ascontiguousarray(X[b][:, cols]).astype(NPDT),
            "mk": mk,
        })
    return in_maps


def assemble(results):
    out = np.empty((B, N, D), dtype=np.float32)
    for core in range(NCORES):
        b, g = divmod(core, 4)
        out[b, :, CW * g:CW * g + CW] = results[core]["outT"].T
    return out


def kernel(hidden_states, queries_weight):
    nc = get_nc()
    in_maps = make_in_maps(hidden_states, queries_weight)
    res = bass_utils.run_bass_kernel_spmd(nc, in_maps, core_ids=list(range(NCORES)))
    return assemble(res.results)


# revision 17
# speedup vs baseline: 1.5739x; 1.5739x over previous
"""DenseAttention (causal quadratic variant, no softmax) — TRN2 Bass kernel.

Problem: out[b] = (tril(Q @ K^T) @ V) per head, where
  Q = X @ Wq (split into 16 heads of 64), K = V = X head slices.
Shapes: X [2, 2048, 1024] fp32, Wq [1024, 1024] fp32 -> out [2, 2048, 1024] fp32.

Sharding (8 cores): core c -> batch b = c//4, head group g = c%4 (4 heads,
output columns [256g, 256g+256)).  The queries projection is column-sharded
by head group; no cross-device communication.

Algorithm per core (linear-attention prefix-sum form, per head h):
  attn_I = Q_I @ S_{<I} + tril(Q_I @ K_I^T) @ V_I      (blocks I of 256 rows)
  S_I = S_{<I} + sum over 128-blocks j in I of K_j^T @ V_j   ([64,64] state)
This reduces the strictly-causal off-diagonal work from O(N^2 hd) to O(N hd^2).
Everything is computed transposed (attnT [hd, N]) so both matmul stages feed
the tensor engine without any on-device transposes; the host un-transposes.

All matmuls run in bf16 with fp32 PSUM accumulation (validated ~2.8e-3 rel
error vs the fp32 reference in numpy emulation).
"""

import numpy as np
import ml_dtypes

import concourse.bacc as bacc
import concourse.mybir as mybir
import concourse.tile as tile
from concourse import bass_utils
from concourse.bass import ds, ts

B, N, D = 2, 2048, 1024
H, HD = 16, 64
NCORES = 8
P = 128           # partition dim
NQ = 256          # q-block (outer) size
T = N // NQ       # 8 outer blocks
KB = N // P       # 16 k-blocks
CW = 256          # per-core output column width (4 heads)

DT = mybir.dt.bfloat16
NPDT = ml_dtypes.bfloat16
F32 = mybir.dt.float32


def _emit(nc, tc, xt_d, wq_d, xv_d, mk_d, out_d):
    with (
        tc.tile_pool(name="const", bufs=1) as cpool,
        tc.tile_pool(name="work", bufs=8) as wpool,
        tc.tile_pool(name="psq", bufs=1, space="PSUM") as psq,
        tc.tile_pool(name="psat", bufs=2, space="PSUM") as psat,
    ):
        # ---------------- input DMAs: few, large, strided transfers.
        # Row-blocks of the DRAM tensors are folded into the free dimension
        # of single wide SBUF tiles ([p, (blk, w)] layout) so each logical
        # input is one DMA.  mask/wq go on the ACT HWDGE queue, xv/xt on the
        # SP queue.  xt is loaded in 512-column chunks, chunk-major, so
        # Q-proj chunk c (and the attention blocks it unlocks) only waits
        # for (c+1)/4 of the xt traffic.
        mk_sb = cpool.tile([P, 2 * NQ], DT, name="mk_sb", tag="mk_sb")
        nc.scalar.dma_start(out=mk_sb, in_=mk_d)

        wqall = cpool.tile([P, 8 * CW], DT, name="wqall", tag="wqall")
        nc.scalar.dma_start(
            out=wqall.rearrange("p (k w) -> p k w", k=8),
            in_=wq_d.rearrange("(k p) w -> p k w", p=P),
        )

        xvall = cpool.tile([P, KB * CW], DT, name="xvall", tag="xvall")
        for h in range(2):
            nc.sync.dma_start(
                out=xvall.rearrange("p (j w) -> p j w", j=KB)[:, ds(8 * h, 8), :],
                in_=xv_d.rearrange("(j p) w -> p j w", p=P)[:, ds(8 * h, 8), :],
            )

        xtall = cpool.tile([P, 8 * N], DT, name="xtall", tag="xtall")
        for c in range(4):
            nc.sync.dma_start(
                out=xtall.rearrange("p (k n) -> p k n", k=8)[:, :, ds(512 * c, 512)],
                in_=xt_d.rearrange("(k p) n -> p k n", p=P)[:, :, ds(512 * c, 512)],
            )

        def xt_ap(k, col, w):
            return xtall[:, ds(N * k + col, w)]

        def xv_ap(j, col, w):
            return xvall[:, ds(CW * j + col, w)]

        # ---------------- S phase: running prefix sums S_t = sum_{j<=2t+1} K_j^T V_j
        # One Gram matmul per (pair, j): X_pair^T @ X_pair [128,128]; the two
        # diagonal 64x64 blocks are the per-head S states, off-diagonal blocks
        # are never read.  Snapshots after each outer block t (t=0..6).
        ssb = [[None] * (T - 1) for _ in range(2)]
        with tc.tile_pool(name="pss", bufs=1, space="PSUM") as pss:
            for p in range(2):
                sps = pss.tile([P, P], F32, name=f"sps{p}", tag=f"sps{p}")
                for j in range(KB):
                    v = xv_ap(j, P * p, P)
                    # skip_group_check: snapshots legitimately read the
                    # partial sum mid-accumulation-group (legal on HW)
                    nc.tensor.matmul(
                        sps, v, v, start=(j == 0), stop=(j == KB - 1),
                        skip_group_check=True,
                    )
                    if j % 2 == 1 and j < KB - 1:
                        t_idx = j // 2
                        snap = cpool.tile(
                            [P, HD], DT, name=f"ssb{p}_{t_idx}", tag=f"ssb{p}_{t_idx}"
                        )
                        for e in range(2):
                            nc.vector.tensor_copy(
                                snap[ds(HD * e, HD), :],
                                sps[ds(HD * e, HD), ds(HD * e, HD)],
                            )
                        ssb[p][t_idx] = snap

            qt_sb = [
                cpool.tile([P, N], DT, name=f"qt{m}", tag=f"qt{m}") for m in range(2)
            ]

        # ---------------- fused main loop over 512-column chunks c:
        #   Q-proj chunk c (both m halves), then attention blocks t=2c, 2c+1.
        # ST scores for both t's are emitted before the PV stage so the PE
        # has independent matmuls while the DVE does masked PSUM->SBUF copies.
        # o=0 block: full [128, 256] (left half tril-masked, right half dense).
        # o=1 block: only the right [128, 128] survives the mask (tril there).
        with tc.tile_pool(name="psst", bufs=4, space="PSUM") as psst:

            def emit_sts(p, t):
                out = []
                for o in range(2):
                    j = 2 * t + o
                    w_ = NQ if o == 0 else P
                    for e in range(2):
                        stp = psst.tile(
                            [P, NQ], F32, name=f"stp{p}_{t}_{o}_{e}", tag="stp"
                        )
                        kt = xtall[ds(HD * e, HD), ds(N * p + P * j, P)]
                        qv = qt_sb[p][ds(HD * e, HD), ds(NQ * t + (NQ - w_), w_)]
                        nc.tensor.matmul(stp[:, :w_], kt, qv, start=True, stop=True)
                        stsb = wpool.tile(
                            [P, NQ], DT, name=f"st{p}_{t}_{o}_{e}", tag="st",
                            bufs=16,
                        )
                        # causal mask fused into the PSUM->SBUF copy; the
                        # o=1 right half sees the same tril pattern as mk[:, :128]
                        mslice = mk_sb[:, :NQ] if o == 0 else mk_sb[:, :P]
                        nc.vector.tensor_mul(stsb[:, :w_], stp[:, :w_], mslice)
                        out.append((o, e, w_, stsb))
                return out

            for c in range(4):
                # Q projection chunk c: qt[m][:, 512c:512c+512] = sum_k ...
                for m in range(2):
                    qp = psq.tile([P, 512], F32, name=f"qp{m}_{c}", tag=f"qp{m}")
                    for k in range(8):
                        nc.tensor.matmul(
                            qp,
                            wqall[:, ds(CW * k + P * m, P)],
                            xt_ap(k, 512 * c, 512),
                            start=(k == 0), stop=(k == 7),
                        )
                    nc.scalar.copy(qt_sb[m][:, ds(512 * c, 512)], qp)

                sts = {}
                for t in (2 * c, 2 * c + 1):
                    for p in range(2):
                        sts[(t, p)] = emit_sts(p, t)

                for t in (2 * c, 2 * c + 1):
                    for p in range(2):
                        at = psat.tile([P, NQ], F32, name=f"at{p}_{t}", tag="at")

                        # global term: attnT_t += S_{<t}^T @ Q_t^T (S symmetric)
                        for e in range(2):
                            if t > 0:
                                nc.tensor.matmul(
                                    at[ds(HD * e, HD), :],
                                    ssb[p][t - 1][ds(HD * e, HD), :],
                                    qt_sb[p][ds(HD * e, HD), ds(NQ * t, NQ)],
                                    start=True, stop=False,
                                    tile_position=(HD * e, HD * e),
                                    # sim's coarse group check mishandles
                                    # base_partition 64 slices; per-partition
                                    # has_written semantics are correct
                                    skip_group_check=True,
                                )

                        # diagonal term: attnT_t += V_j^T @ ST_j
                        for o, e, w_, stsb in sts[(t, p)]:
                            j = 2 * t + o
                            nc.tensor.matmul(
                                at[ds(HD * e, HD), ds(NQ - w_, w_)],
                                xv_ap(j, P * p + HD * e, HD),
                                stsb[:, :w_],
                                start=(t == 0 and o == 0), stop=(o == 1),
                                tile_position=(0, HD * e),
                                skip_group_check=True,
                            )

                        ot = wpool.tile([P, NQ], F32, name=f"ot{p}_{t}", tag="ot")
                        nc.scalar.copy(ot, at)
                        nc.sync.dma_start(
                            out=out_d[ds(P * p, P), ds(NQ * t, NQ)], in_=ot
                        )


def build_nc(loop_n=1):
    nc = bacc.Bacc("TRN2", target_bir_lowering=False, debug=False)
    xt_d = nc.dram_tensor("xt", [D, N], DT, kind="ExternalInput").ap()
    wq_d = nc.dram_tensor("wq", [D, CW], DT, kind="ExternalInput").ap()
    xv_d = nc.dram_tensor("xv", [N, CW], DT, kind="ExternalInput").ap()
    mk_d = nc.dram_tensor("mk", [P, 2 * NQ], DT, kind="ExternalInput").ap()
    out_d = nc.dram_tensor("outT", [CW, N], F32, kind="ExternalOutput").ap()
    with tile.TileContext(nc) as tc:
        if loop_n > 1:
            # timing-only build: repeat the whole kernel on-device so the
            # per-iteration time can be separated from host/RPC overhead
            with tc.For_i(0, loop_n, 1):
                _emit(nc, tc, xt_d, wq_d, xv_d, mk_d, out_d)
        else:
            _emit(nc, tc, xt_d, wq_d, xv_d, mk_d, out_d)
    nc.compile()
    return nc


_CACHE = {}


def get_nc():
    if "nc" not in _CACHE:
        _CACHE["nc"] = build_nc()
    return _CACHE["nc"]


def make_in_maps(hidden_states, queries_weight):
    X = np.asarray(hidden_states, dtype=np.float32)
    W = np.asarray(queries_weight, dtype=np.float32)
    r = np.arange(P)[:, None]
    c = np.arange(NQ)[None, :]
    m0 = (c >= r).astype(np.float32)
    m1 = (c >= r + P).astype(np.float32)
    mk = np.concatenate([m0, m1], axis=1).astype(NPDT)
    in_maps = []
    for core in range(NCORES):
        b, g = divmod(core, 4)
        cols = slice(CW * g, CW * g + CW)
        # Permute the contraction rows so every core sees its own heads'
        # K^T rows at xt rows [0, 256) (keeps the program core-agnostic).
        perm = np.r_[
            np.arange(CW * g, CW * g + CW),
            np.arange(0, CW * g),
            np.arange(CW * g + CW, D),
        ]
        in_maps.append({
            "xt": np.ascontiguousarray(X[b].T[perm]).astype(NPDT),
            "wq": np.ascontiguousarray(W[perm][:, cols]).astype(NPDT),
            "xv": np.ascontiguousarray(X[b][:, cols]).astype(NPDT),
            "mk": mk,
        })
    return in_maps


def assemble(results):
    out = np.empty((B, N, D), dtype=np.float32)
    for core in range(NCORES):
        b, g = divmod(core, 4)
        out[b, :, CW * g:CW * g + CW] = results[core]["outT"].T
    return out


def kernel(hidden_states, queries_weight):
    nc = get_nc()
    in_maps = make_in_maps(hidden_states, queries_weight)
    res = bass_utils.run_bass_kernel_spmd(nc, in_maps, core_ids=list(range(NCORES)))
    return assemble(res.results)
